# revision 3
# baseline (speedup 1.0000x reference)
"""Multi-head attention (B=2, S=2048, E=1024, H=16) on 8 TRN2 NeuronCores.

Sharding: core c handles batch b = c // 4 and the 4 heads
[4*(c%4), 4*(c%4)+4). Each core computes its heads' Q/K/V projections,
attention probs (written fp32, natural [s_q, s_k] layout for contiguous
DMA), context, and a partial out-projection [S, E]. The host sums the 4
partial outputs per batch and concatenates probs.

Self-contained: hardcodes all shapes; no sibling imports.
"""
import sys, os, types, ctypes, contextlib, functools

sys.path.insert(0, "/opt/trn_rl_repo")
os.environ.setdefault("JAX_PLATFORMS", "")

import numpy as np

# --- shim: antenv.axon_hooks (missing in this image; needed by
# run_bass_kernel_spmd(trace=True) under axon) --------------------------
def _ntff_profile_via_ctypes(so_path):
    try:
        lib = ctypes.CDLL(so_path)
    except OSError:
        return None
    if not hasattr(lib, "axon_start_nrt_profile"):
        return None
    lib.axon_start_nrt_profile.argtypes = [ctypes.POINTER(ctypes.c_int64), ctypes.c_size_t]
    lib.axon_start_nrt_profile.restype = ctypes.c_int64
    lib.axon_stop_nrt_profile.argtypes = [ctypes.c_char_p]
    lib.axon_stop_nrt_profile.restype = ctypes.c_int64

    @contextlib.contextmanager
    def _hook(output_dir, device_ids):
        import jax
        jax.devices()
        if device_ids:
            ids = (ctypes.c_int64 * len(device_ids))(*device_ids)
            rc = lib.axon_start_nrt_profile(ids, len(device_ids))
        else:
            rc = lib.axon_start_nrt_profile(None, 0)
        if rc != 0:
            raise RuntimeError(f"axon_start_nrt_profile rc={rc}")
        try:
            yield
        finally:
            n = lib.axon_stop_nrt_profile(str(output_dir).encode())
            print(f"profile: {n} file(s) written to {output_dir}", file=sys.stderr)

    return _hook


if "antenv.axon_hooks" not in sys.modules:
    _mod = types.ModuleType("antenv.axon_hooks")
    _hook = _ntff_profile_via_ctypes("/opt/axon/libaxon_pjrt.so")
    _mod.get_axon_ntff_profile_hook = lambda: _hook
    _mod.set_axon_ntff_profile_hook = lambda h: None
    sys.modules["antenv.axon_hooks"] = _mod
# ----------------------------------------------------------------------

from contextlib import ExitStack

import concourse.bacc as bacc
import concourse.tile as tile
from concourse import mybir
import concourse.bass_utils as bass_utils
from concourse.bass_utils import run_bass_kernel_spmd

bass_utils.upload_artifacts = lambda tmpdir: f"file://{tmpdir}"  # no artifact bucket here

B, S, E = 2, 2048, 1024
H, D = 16, 64
HPC = 4            # heads per core
N_CORES = 8
SCALE = 1.0 / 8.0  # 1/sqrt(D)

F32 = mybir.dt.float32
F32R = mybir.dt.float32r
BF16 = mybir.dt.bfloat16


def r(ap):
    """Operands feeding FP32r matmuls are already float32r-typed."""
    return ap


def build_nc():
    nc = bacc.Bacc(trn_type="TRN2", target_bir_lowering=False, debug=False)

    ht = nc.dram_tensor("ht", [E, S], F32R, kind="ExternalInput").ap()
    wq = nc.dram_tensor("wq", [E, HPC * D], F32R, kind="ExternalInput").ap()
    wk = nc.dram_tensor("wk", [E, HPC * D], F32R, kind="ExternalInput").ap()
    wv = nc.dram_tensor("wv", [E, HPC * D], F32R, kind="ExternalInput").ap()
    wo = nc.dram_tensor("wo", [HPC * D, E], F32R, kind="ExternalInput").ap()
    bq = nc.dram_tensor("bq", [2, 128], F32, kind="ExternalInput").ap()
    bk = nc.dram_tensor("bk", [2, 128], F32, kind="ExternalInput").ap()
    bv_bc = nc.dram_tensor("bv_bc", [128, HPC * D], F32, kind="ExternalInput").ap()
    pattern = nc.dram_tensor("pattern", [2, 128], F32, kind="ExternalInput").ap()
    ident = nc.dram_tensor("ident", [128, 128], F32, kind="ExternalInput").ap()

    probs = nc.dram_tensor("probs", [HPC, S, S], F32, kind="ExternalOutput").ap()
    pout = nc.dram_tensor("pout", [S, E], F32, kind="ExternalOutput").ap()

    KT = E // 128      # 8 contraction k-tiles for projections
    NSQ = S // 512     # 4 s_q slices of 512
    NST = S // 128     # 16 s tiles of 128
    NKT = S // 128     # 16 s_k tiles of 128

    with tile.TileContext(nc) as tc, ExitStack() as ctx:
        # ---------------- pools ----------------
        p_ht = ctx.enter_context(tc.tile_pool(name="ht", bufs=KT))
        p_w = ctx.enter_context(tc.tile_pool(name="w", bufs=3 * KT))
        p_wo = ctx.enter_context(tc.tile_pool(name="wo", bufs=2))
        p_qk = ctx.enter_context(tc.tile_pool(name="qk", bufs=4))
        p_v = ctx.enter_context(tc.tile_pool(name="v", bufs=NST))
        p_expT = ctx.enter_context(tc.tile_pool(name="expT", bufs=6))
        p_e = ctx.enter_context(tc.tile_pool(name="e", bufs=2))
        p_ctxn = ctx.enter_context(tc.tile_pool(name="ctxn", bufs=2))
        p_outsb = ctx.enter_context(tc.tile_pool(name="outsb", bufs=2))
        p_small = ctx.enter_context(tc.tile_pool(name="small", bufs=1))
        p_sums = ctx.enter_context(tc.tile_pool(name="sums", bufs=12))
        p_rsb = ctx.enter_context(tc.tile_pool(name="rsb", bufs=2))

        ps_big = ctx.enter_context(tc.tile_pool(name="ps_big", bufs=2, space="PSUM"))
        ps_a = ctx.enter_context(tc.tile_pool(name="ps_a", bufs=2, space="PSUM"))
        ps_ctx = ctx.enter_context(tc.tile_pool(name="ps_ctx", bufs=1, space="PSUM"))
        ps_small = ctx.enter_context(tc.tile_pool(name="ps_small", bufs=1, space="PSUM"))

        # ---------------- constants / weights in ----------------
        ht_t = []
        for k in range(KT):
            t = p_ht.tile([128, S], F32R, tag="ht", name="ht_t")
            nc.sync.dma_start(t[:], ht[k * 128:(k + 1) * 128, :])
            ht_t.append(t)

        wq_t, wk_t, wv_t = [], [], []
        for name, dram, lst in (("wq", wq, wq_t), ("wk", wk, wk_t), ("wv", wv, wv_t)):
            for k in range(KT):
                t = p_w.tile([128, HPC * D], F32R, tag="w", name="w_t")
                nc.sync.dma_start(t[:], dram[k * 128:(k + 1) * 128, :])
                lst.append(t)

        wo_t = []
        for p in range(2):
            t = p_wo.tile([128, E], F32R, tag="wo", name="wo_t")
            nc.sync.dma_start(t[:], wo[p * 128:(p + 1) * 128, :])
            wo_t.append(t)

        bq_t, bk_t = [], []
        for p in range(2):
            t = p_small.tile([128, 1], F32, tag=f"bq{p}")
            nc.sync.dma_start(t[:], bq[p, :])
            bq_t.append(t)
            t = p_small.tile([128, 1], F32, tag=f"bk{p}")
            nc.sync.dma_start(t[:], bk[p, :])
            bk_t.append(t)
        bv_t = p_small.tile([128, HPC * D], F32, tag="bv")
        nc.sync.dma_start(bv_t[:], bv_bc[:, :])
        pat_t = p_small.tile([2, 128], F32, tag="pat")
        nc.sync.dma_start(pat_t[:], pattern[:, :])
        id_t = p_small.tile([128, 128], F32, tag="ident")
        nc.sync.dma_start(id_t[:], ident[:, :])

        # ---------------- phase 1: projections ----------------
        # qT/kT: [128 (pair d-dims), S] = W_slice.T @ hT  (+ bias per-partition)
        qT, kT = [], []
        for p in range(2):
            qT.append(p_qk.tile([128, S], F32R, tag="qk", name="qk_t"))
            kT.append(p_qk.tile([128, S], F32R, tag="qk", name="qk_t"))
        for p in range(2):
            for (w_t, dst, b_t) in ((wq_t, qT[p], bq_t[p]), (wk_t, kT[p], bk_t[p])):
                for sq in range(NSQ):
                    ps = ps_big.tile([128, 1024], F32, tag="ps_big", name="psb")
                    for k in range(KT):
                        nc.tensor.matmul(
                            ps[:, 0:512],
                            r(w_t[k][:, p * 128:(p + 1) * 128]),
                            r(ht_t[k][:, sq * 512:(sq + 1) * 512]),
                            start=(k == 0), stop=(k == KT - 1),
                        )
                    nc.vector.tensor_scalar_add(
                        dst[:, sq * 512:(sq + 1) * 512], ps[:, 0:512], b_t[:]
                    )

        # v natural: [s-tile 128, HPC*D] bf16 = hT_tile.T @ wv (+ bv broadcast)
        v_t = []
        for st in range(NST):
            ps = ps_big.tile([128, 1024], F32, tag="ps_big", name="psb")
            for k in range(KT):
                nc.tensor.matmul(
                    ps[:, 0:HPC * D],
                    r(ht_t[k][:, st * 128:(st + 1) * 128]),
                    r(wv_t[k][:, :]),
                    start=(k == 0), stop=(k == KT - 1),
                )
            vt = p_v.tile([128, HPC * D], BF16, tag="v", name="v_tl")
            nc.vector.tensor_tensor(
                vt[:], ps[:, 0:HPC * D], bv_t[:], op=mybir.AluOpType.add
            )
            v_t.append(vt)

        # ---------------- phase 2: attention ----------------
        ctxn = [p_ctxn.tile([128, S], F32R, tag="ctxn", name="ctxn_t") for _ in range(2)]
        for p in range(2):
            hA, hB = 2 * p, 2 * p + 1
            for sq in range(NSQ):
                sq_lo = sq * 512
                # --- side A: scoresT -> expT(bf16) -> ctx accumulate ---
                ctx_ps = ps_ctx.tile([128, 512], F32, tag="ps_ctx", name="ctx_ps")
                for kt in range(NKT):
                    sA = ps_a.tile([128, 512], F32, tag="ps_a", name="s_ps")
                    sB = ps_a.tile([128, 512], F32, tag="ps_a", name="s_ps")
                    nc.tensor.matmul(
                        sA[:],
                        r(kT[p][0:64, kt * 128:(kt + 1) * 128]),
                        r(qT[p][0:64, sq_lo:sq_lo + 512]),
                        tile_position=(0, 0),
                    )
                    nc.tensor.matmul(
                        sB[:],
                        r(kT[p][64:128, kt * 128:(kt + 1) * 128]),
                        r(qT[p][64:128, sq_lo:sq_lo + 512]),
                        tile_position=(64, 0),
                    )
                    eA = p_expT.tile([128, 512], BF16, tag="expT", name="expT_t")
                    eB = p_expT.tile([128, 512], BF16, tag="expT", name="expT_t")
                    nc.scalar.activation(eA[:], sA[:], mybir.ActivationFunctionType.Exp, scale=SCALE)
                    nc.scalar.activation(eB[:], sB[:], mybir.ActivationFunctionType.Exp, scale=SCALE)
                    nc.tensor.matmul(
                        ctx_ps[0:64, :],
                        v_t[kt][:, hA * 64:(hA + 1) * 64],
                        eA[:],
                        start=(kt == 0), stop=(kt == NKT - 1),
                        tile_position=(0, 0),
                    )
                    nc.tensor.matmul(
                        ctx_ps[64:128, :],
                        v_t[kt][:, hB * 64:(hB + 1) * 64],
                        eB[:],
                        start=(kt == 0), stop=(kt == NKT - 1),
                        tile_position=(0, 64),
                    )

                # --- side B: natural scores -> probs (fp32) + sums ---
                sums2 = []
                for t in range(4):
                    st = sq * 4 + t
                    s2 = p_sums.tile([128, 2], F32, tag="sums2", name="sums2_t")
                    sums2.append(s2)
                    for hi, h in enumerate((hA, hB)):
                        lo, hi_p = (0, 64) if hi == 0 else (64, 128)
                        e_sb = p_e.tile([128, S], F32, tag="e", name="e_sb")
                        sums4 = p_sums.tile([128, 2], F32, tag="sums4", name="sums4_t")
                        for half in range(2):
                            ps = ps_big.tile([128, 1024], F32, tag="ps_big", name="psb")
                            for ck in range(2):
                                nc.tensor.matmul(
                                    ps[:, ck * 512:(ck + 1) * 512],
                                    r(qT[p][lo:hi_p, st * 128:(st + 1) * 128]),
                                    r(kT[p][lo:hi_p, half * 1024 + ck * 512:half * 1024 + (ck + 1) * 512]),
                                    tile_position=(lo, 0),
                                )
                            nc.scalar.activation(
                                e_sb[:, half * 1024:(half + 1) * 1024],
                                ps[:],
                                mybir.ActivationFunctionType.Exp,
                                scale=SCALE,
                                accum_out=sums4[:, half:half + 1],
                            )
                        nc.vector.reduce_sum(s2[:, hi:hi + 1], sums4[:], axis=mybir.AxisListType.X)
                        rs = p_sums.tile([128, 1], F32, tag="rs", name="rs_t")
                        nc.vector.reciprocal(rs[:], s2[:, hi:hi + 1])
                        nc.vector.tensor_scalar_mul(e_sb[:], e_sb[:], rs[:])
                        nc.sync.dma_start(probs[h, st * 128:(st + 1) * 128, :], e_sb[:])

                # --- R = 1 / broadcast(sums) ; ctx normalize ---
                rT_ps = ps_small.tile([128, 512], F32, tag="ps_small", name="ps_sm")
                for t in range(4):
                    nc.tensor.matmul(
                        rT_ps[0:2, t * 128:(t + 1) * 128],
                        sums2[t][:],
                        id_t[:],
                    )
                rT_sb = p_rsb.tile([2, 512], F32, tag="rT", name="rT_sb")
                nc.vector.tensor_copy(rT_sb[:], rT_ps[0:2, :])
                R_ps = ps_small.tile([128, 512], F32, tag="ps_small", name="ps_sm")
                for t in range(4):
                    nc.tensor.matmul(
                        R_ps[:, t * 128:(t + 1) * 128],
                        pat_t[:],
                        rT_sb[:, t * 128:(t + 1) * 128],
                    )
                R_sb = p_rsb.tile([128, 512], F32, tag="R", name="R_sb")
                nc.vector.reciprocal(R_sb[:], R_ps[:])
                nc.vector.tensor_tensor(
                    ctxn[p][0:64, sq_lo:sq_lo + 512], ctx_ps[0:64, :], R_sb[0:64, :],
                    op=mybir.AluOpType.mult,
                )
                nc.vector.tensor_tensor(
                    ctxn[p][64:128, sq_lo:sq_lo + 512], ctx_ps[64:128, :], R_sb[64:128, :],
                    op=mybir.AluOpType.mult,
                )

        # ---------------- phase 3: out projection ----------------
        for st in range(NST):
            o_sb = p_outsb.tile([128, E], F32, tag="outsb", name="o_sb")
            for eo in range(2):
                po = ps_small.tile([128, 512], F32, tag="ps_small", name="ps_sm")
                for p in range(2):
                    nc.tensor.matmul(
                        po[:],
                        r(ctxn[p][:, st * 128:(st + 1) * 128]),
                        r(wo_t[p][:, eo * 512:(eo + 1) * 512]),
                        start=(p == 0), stop=(p == 1),
                    )
                nc.vector.tensor_copy(o_sb[:, eo * 512:(eo + 1) * 512], po[:])
            nc.sync.dma_start(pout[st * 128:(st + 1) * 128, :], o_sb[:])

    nc.compile()
    return nc


@functools.lru_cache(maxsize=1)
def _get_nc():
    return build_nc()


def _make_in_maps(hidden_states, Wq, bq, Wk, bk, Wv, bv, Wo, bo):
    pattern = np.zeros((2, 128), np.float32)
    pattern[0, 0:64] = 1.0
    pattern[1, 64:128] = 1.0
    ident = np.eye(128, dtype=np.float32)

    in_maps = []
    for c in range(N_CORES):
        b = c // 4
        h0 = (c % 4) * HPC
        sl = slice(h0 * D, (h0 + HPC) * D)
        in_maps.append({
            "ht": np.ascontiguousarray(hidden_states[b].T),
            "wq": np.ascontiguousarray(Wq[:, sl]),
            "wk": np.ascontiguousarray(Wk[:, sl]),
            "wv": np.ascontiguousarray(Wv[:, sl]),
            "wo": np.ascontiguousarray(Wo[sl, :]),
            "bq": np.ascontiguousarray(bq[sl]).reshape(2, 128),
            "bk": np.ascontiguousarray(bk[sl]).reshape(2, 128),
            "bv_bc": np.broadcast_to(bv[sl], (128, HPC * D)).copy(),
            "pattern": pattern,
            "ident": ident,
        })
    return in_maps


def kernel(hidden_states, Wq, bq, Wk, bk, Wv, bv, Wo, bo, _trace=False, _trace_kwargs=None):
    hidden_states = np.asarray(hidden_states, np.float32)
    Wq, bq = np.asarray(Wq, np.float32), np.asarray(bq, np.float32)
    Wk, bk = np.asarray(Wk, np.float32), np.asarray(bk, np.float32)
    Wv, bv = np.asarray(Wv, np.float32), np.asarray(bv, np.float32)
    Wo, bo = np.asarray(Wo, np.float32), np.asarray(bo, np.float32)

    nc = _get_nc()
    in_maps = _make_in_maps(hidden_states, Wq, bq, Wk, bk, Wv, bv, Wo, bo)
    res = run_bass_kernel_spmd(
        nc, in_maps, list(range(N_CORES)), trace=_trace, **(_trace_kwargs or {})
    )

    probs = np.empty((B, H, S, S), np.float32)
    out = np.zeros((B, S, E), np.float32)
    for c in range(N_CORES):
        b = c // 4
        h0 = (c % 4) * HPC
        probs[b, h0:h0 + HPC] = res.results[c]["probs"]
        out[b] += res.results[c]["pout"]
    out += bo

    kernel.last_exec_time_ns = res.exec_time_ns
    return out, probs
